# revision 37
# baseline (speedup 1.0000x reference)
"""Trainium2 Bass kernel for nn_CtcBoundaryLossV3.

Reference computation (per sample b, T=2048 frames, V=1024 vocab, U=256):
  blank = ctc_log_probs[b, :, 0]
  spike[t] = (blank[t] < log(0.3)) & mask[t]
  pos = sorted spike positions; seg_j = sum(alpha[pos_j .. pos_{j+1}]) (both ends
  inclusive); boundary_j = seg_j if j < n_spikes-1 else 0
  loss = sum_b sum_{j < min(text_len_b, 256)} |boundary_j - 1| / B

Reformulated without sort/scatter (validated vs the jax reference):
  w[t] = A_t * w[t-1] + B_t   with A_t = 1 - spike[t-1],
                                   B_t = alpha[t] + spike[t-1]*alpha[t-1]
  (w at a spike t equals the interval sum ending at t, both ends inclusive)
  c[t] = inclusive cumsum of spike (spike rank)
  loss_b = sum_t |w[t]-1| * spike[t] * (2 <= c[t] <= lim_b)
           + relu(lim_b - 1 - relu(nsp_b - 1))        # invalid slots count as |0-1|
  where lim_b = min(text_len_b, 256) + 1, nsp_b = total spikes.

Device layout (per core, 2 samples): [128, 32] tiles; partition p = s*64 + q,
column c, t = q*32 + c. Scans run two-level: per-block (free-dim)
tensor_tensor_scan, then cross-block combination with PE matmuls against a
host-built constant tensor holding
  W[k,m]    = (k < m) & same-sample        (exclusive-prefix operator)
  SHIFT[k,m]= (m == k+1) & same-sample     (fetch previous block's last column)
plus per-block/per-sample lim columns, sample-selector columns, and I2.
The affine cross-block scan S_m = A_m S_{m-1} + B_m uses A in {0,1}:
  S_excl[m] = sum_k B_k * [no reset in blocks (k, m)] (same sample, k < m)
            = (W * eq).T @ B,  eq[k,m] = (cumRsh[m] == cumRsh[k] + R[k]),
  with R = 1-A (block-has-reset), cumRsh = W.T @ R broadcast down partitions
  via ONES.T @ (W*R) (ONES is a memset bf16 tile; counts are exact in bf16).

DMA placement (4 input DMAs total): the strided blank gather is issued first
on the SP HWDGE ring (it gates the whole compute chain), mask rides behind
it; alpha and the constant tensor go on the ACT HWDGE ring so dispatch costs
overlap instead of serializing on one sequencer. The constant tensor also
carries the pre-shifted alpha[t-1] block-boundary column (host layout prep),
so no extra boundary gathers are needed.

Sharding: pure data parallel, B=16 over 8 cores (2 samples/core). Per-core
output = per-sample losses [2, 1]; host sums and divides by B.
"""
import math
from contextlib import ExitStack

import numpy as np

import concourse.bacc as bacc
import concourse.tile as tile
from concourse import mybir
from concourse.bass_utils import run_bass_kernel_spmd

f32 = mybir.dt.float32
u8 = mybir.dt.uint8
Alu = mybir.AluOpType
Act = mybir.ActivationFunctionType

N_CORES = 8
B_FULL, T, V, U = 16, 2048, 1024, 256
B_LOC = B_FULL // N_CORES  # 2 samples per core
NB = 64    # blocks per sample
BC = 32    # columns (t) per block
P = 128    # partitions = 2 samples * NB
LOG_THRESH = math.log(1.0 - 0.7)  # log(0.3); compared in f32 on device (as in jax)

# consts tensor column layout
C_W = 0          # [0, 128)   W
C_SH = 128       # [128, 256) SHIFT
C_LIMCOL = 256   # col 256    per-block lim
C_SEL = 257      # [257, 259) per-sample column selectors
C_LIM2M1 = 259   # col 259 rows 0:2 = lim - 1 per sample
C_EYE = 260      # [260, 262) rows 0:2 = I2 (fold corr into PSUM accumulation)
C_APREV = 262    # col 262: alpha[t-1] at each block start (pre-shifted layout)
C_NCOLS = 263


def _body(ctx, tc, alpha_d, ctc_d, mask_d, consts_d, out_d):
    nc = tc.nc
    pool = ctx.enter_context(tc.tile_pool(name="p", bufs=1))
    psum = ctx.enter_context(tc.tile_pool(name="ps", bufs=1, space="PSUM"))

    # ---- DMAs. SP ring: the long strided blank gather FIRST (it gates the
    # whole compute chain), then the tiny mask. ACT ring: alpha, then consts
    # (which also carries the pre-shifted alpha[t-1] block-boundary column).
    blank = pool.tile([P, BC], f32)
    mask_sb = pool.tile([P, BC], u8)
    alpha = pool.tile([P, BC], f32)
    consts = pool.tile([P, C_NCOLS], f32)
    alpha_prev = pool.tile([P, BC], f32)
    mask_r = mask_d.rearrange("s (q c) -> (s q) c", c=BC)
    alpha_r = alpha_d.rearrange("s (q c) -> (s q) c", c=BC)
    blank_r = ctc_d[:, :, 0].rearrange("s (q c) -> (s q) c", c=BC)

    nc.sync.dma_start(out=blank[:], in_=blank_r)
    nc.sync.dma_start(out=mask_sb[:], in_=mask_r)
    nc.scalar.dma_start(out=alpha[:], in_=alpha_r)
    nc.scalar.dma_start(out=consts[:], in_=consts_d[:])

    wmat = consts[:, C_W : C_W + P]
    shiftm = consts[:, C_SH : C_SH + P]
    limcol = consts[:, C_LIMCOL : C_LIMCOL + 1]
    selsmp = consts[:, C_SEL : C_SEL + B_LOC]
    lim2m1 = consts[0:B_LOC, C_LIM2M1 : C_LIM2M1 + 1]
    eye2 = consts[0:B_LOC, C_EYE : C_EYE + B_LOC]

    # bf16 ones for the (integer-valued, exact) broadcast matmul
    ones_bf = pool.tile([P, P], mybir.dt.bfloat16)
    nc.gpsimd.memset(ones_bf[:], 1.0)

    # ---- ACT-side prep (all off the blank critical path) ----
    nc.scalar.copy(out=alpha_prev[:, 1:BC], in_=alpha[:, 0 : BC - 1])
    nc.scalar.copy(out=alpha_prev[:, 0:1], in_=consts[:, C_APREV : C_APREV + 1])
    # am1 = alpha - 1 (folded into the |w-1| term later)
    am1 = pool.tile([P, BC], f32)
    nc.scalar.activation(out=am1[:], in_=alpha[:], func=Act.Copy, bias=-1.0, scale=1.0)

    # ---- DVE chain ----
    # spike = (blank < thresh) * mask
    spike = pool.tile([P, BC], f32)
    nc.vector.scalar_tensor_tensor(
        out=spike[:], in0=blank[:], scalar=LOG_THRESH, in1=mask_sb[:],
        op0=Alu.is_lt, op1=Alu.mult,
    )
    # within-block spike count scan
    clocal = pool.tile([P, BC], f32)
    nc.vector.tensor_tensor_scan(out=clocal[:], data0=spike[:], data1=spike[:],
                                 initial=0.0, op0=Alu.add, op1=Alu.bypass)
    # previous-element spike column via PE shift
    spsh = psum.tile([P, 1], f32)
    nc.tensor.matmul(spsh[:], shiftm, spike[:, BC - 1 : BC], start=True, stop=True)
    # R[p] = block has >=1 reset = (clocal[:,30] + spike_prev) >= 1 (fused).
    # Emitted before the a0 ops: the S-path (Rcol -> WR -> X -> Mp -> Sexcl)
    # is the longest dependency chain, so it gets the DVE first once the PE
    # shift lands.
    Rcol = pool.tile([P, 1], f32)
    nc.vector.tensor_scalar(out=Rcol[:], in0=clocal[:, BC - 2 : BC - 1],
                            scalar1=spsh[:], scalar2=1.0, op0=Alu.add,
                            op1=Alu.is_ge)
    # cross-block no-reset-in-(k,m) operator: Mp = W * (ONES.T@(W*R) == W.T@R + R)
    WR = pool.tile([P, P], mybir.dt.bfloat16)
    nc.vector.tensor_scalar(out=WR[:], in0=wmat, scalar1=Rcol[:], scalar2=None,
                            op0=Alu.mult)
    # a0[t] = 1 - spike[t-1]
    a0 = pool.tile([P, BC], f32)
    nc.vector.tensor_scalar(out=a0[:, 1:BC], in0=spike[:, 0 : BC - 1],
                            scalar1=-1.0, scalar2=1.0, op0=Alu.mult, op1=Alu.add)
    nc.vector.tensor_scalar(out=a0[:, 0:1], in0=spsh[:],
                            scalar1=-1.0, scalar2=1.0, op0=Alu.mult, op1=Alu.add)
    # level-1 scans: v[t] = (1-spike[t-1])*v[t-1] + alpha[t-1]  (v = w - alpha)
    vloc = pool.tile([P, BC], f32)
    nc.vector.tensor_tensor_scan(out=vloc[:], data0=a0[:], data1=alpha_prev[:],
                                 initial=0.0, op0=Alu.mult, op1=Alu.add)
    ploc = pool.tile([P, BC], f32)
    nc.vector.tensor_tensor_scan(out=ploc[:], data0=a0[:], data1=a0[:],
                                 initial=1.0, op0=Alu.mult, op1=Alu.bypass)
    crcol = psum.tile([P, 1], f32)
    nc.tensor.matmul(crcol[:], wmat, Rcol[:], start=True, stop=True)
    X = psum.tile([P, P], f32)
    nc.tensor.matmul(X[:], ones_bf[:], WR[:], start=True, stop=True)
    ek = pool.tile([P, 1], f32)
    nc.vector.tensor_add(ek[:], crcol[:], Rcol[:])
    Mp = pool.tile([P, P], f32)
    nc.vector.scalar_tensor_tensor(out=Mp[:], in0=X[:], scalar=ek[:], in1=wmat,
                                   op0=Alu.is_equal, op1=Alu.mult)
    Sexcl = psum.tile([P, 1], f32)
    nc.tensor.matmul(Sexcl[:], Mp[:], vloc[:, BC - 1 : BC], start=True, stop=True)

    # rank gate, computed in parallel with the S-path:
    # g2 = spike * (2 <= rank <= lim), rank = clocal + Cexcl
    Cexcl = psum.tile([P, 1], f32)
    nc.tensor.matmul(Cexcl[:], wmat, clocal[:, BC - 1 : BC], start=True, stop=True)
    rfull = pool.tile([P, BC], f32)
    nc.vector.tensor_scalar(out=rfull[:], in0=clocal[:], scalar1=Cexcl[:],
                            scalar2=None, op0=Alu.add)
    g1 = pool.tile([P, BC], f32)
    nc.vector.scalar_tensor_tensor(out=g1[:], in0=rfull[:], scalar=2.0,
                                   in1=spike[:], op0=Alu.is_ge, op1=Alu.mult)
    g2 = pool.tile([P, BC], f32)
    nc.vector.scalar_tensor_tensor(out=g2[:], in0=rfull[:], scalar=limcol,
                                   in1=g1[:], op0=Alu.is_le, op1=Alu.mult)
    # pre = vloc + alpha - 1 (so w - 1 = ploc*Sexcl + pre)
    pre = pool.tile([P, BC], f32)
    nc.vector.tensor_add(pre[:], vloc[:], am1[:])

    # w - 1 at spikes, gated; |x*g| = |x|*g since g >= 0
    w1 = pool.tile([P, BC], f32)
    nc.vector.scalar_tensor_tensor(out=w1[:], in0=ploc[:], scalar=Sexcl[:],
                                   in1=pre[:], op0=Alu.mult, op1=Alu.add)
    m = pool.tile([P, BC], f32)
    nc.vector.tensor_mul(m[:], w1[:], g2[:])
    s3 = pool.tile([P, BC], f32)
    part = pool.tile([P, 1], f32)
    nc.scalar.activation(out=s3[:], in_=m[:], func=Act.Abs, accum_out=part[:])

    # ---- per-sample correction on the (otherwise idle) Pool engine ----
    nsp2 = psum.tile([B_LOC, 1], f32)
    nc.tensor.matmul(nsp2[:], selsmp, clocal[:, BC - 1 : BC], start=True, stop=True)

    r1 = pool.tile([B_LOC, 1], f32)
    nc.vector.tensor_scalar(out=r1[:], in0=nsp2[:], scalar1=-1.0, scalar2=0.0,
                            op0=Alu.add, op1=Alu.max)
    r2 = pool.tile([B_LOC, 1], f32)
    nc.vector.scalar_tensor_tensor(out=r2[:], in0=r1[:], scalar=-1.0, in1=lim2m1,
                                   op0=Alu.mult, op1=Alu.add)
    corr2 = pool.tile([B_LOC, 1], f32)
    nc.vector.tensor_scalar(out=corr2[:], in0=r2[:], scalar1=0.0, scalar2=None,
                            op0=Alu.max)

    # ---- per-sample totals: PSUM-accumulate block sums + correction ----
    tot2 = psum.tile([B_LOC, 1], f32)
    nc.tensor.matmul(tot2[:], selsmp, part[:], start=True, stop=False)
    nc.tensor.matmul(tot2[:], eye2, corr2[:], start=False, stop=True)
    total = pool.tile([B_LOC, 1], f32)
    nc.scalar.copy(out=total[:], in_=tot2[:])
    nc.sync.dma_start(out=out_d[:], in_=total[:])


def build_nc():
    nc = bacc.Bacc("TRN2", target_bir_lowering=False, debug=False, num_devices=N_CORES)
    alpha_d = nc.dram_tensor("alpha", [B_LOC, T], f32, kind="ExternalInput")
    ctc_d = nc.dram_tensor("ctc", [B_LOC, T, V], f32, kind="ExternalInput")
    mask_d = nc.dram_tensor("mask", [B_LOC, T], u8, kind="ExternalInput")
    consts_d = nc.dram_tensor("consts", [P, C_NCOLS], f32, kind="ExternalInput")
    out_d = nc.dram_tensor("out", [B_LOC, 1], f32, kind="ExternalOutput")
    with tile.TileContext(nc) as tc:
        with ExitStack() as ctx:
            _body(ctx, tc, alpha_d.ap(), ctc_d.ap(), mask_d.ap(), consts_d.ap(),
                  out_d.ap())
    nc.compile()
    return nc


_NC_CACHE = None


def _get_nc():
    global _NC_CACHE
    if _NC_CACHE is None:
        _NC_CACHE = build_nc()
    return _NC_CACHE


def _make_consts(lim_loc, alpha_loc):
    k = np.arange(P)
    same = (k[:, None] // NB) == (k[None, :] // NB)
    consts = np.zeros((P, C_NCOLS), np.float32)
    consts[:, C_W : C_W + P] = ((k[:, None] < k[None, :]) & same)
    consts[:, C_SH : C_SH + P] = ((k[None, :] == k[:, None] + 1) & same)
    consts[:, C_LIMCOL] = np.repeat(lim_loc, NB)
    consts[:NB, C_SEL] = 1.0
    consts[NB:, C_SEL + 1] = 1.0
    consts[0:B_LOC, C_LIM2M1] = lim_loc - 1.0
    consts[0:B_LOC, C_EYE : C_EYE + B_LOC] = np.eye(B_LOC, dtype=np.float32)
    # alpha at t = q*32 - 1 for partition p = s*64 + q (0 at sample starts)
    ap = alpha_loc.reshape(B_LOC, NB, BC)[:, :, BC - 1]  # last col of each block
    col = np.zeros((B_LOC, NB), np.float32)
    col[:, 1:] = ap[:, : NB - 1]
    consts[:, C_APREV] = col.reshape(P)
    return consts


def make_in_maps(alpha, ctc_log_probs, mask, text_length):
    lim_full = (np.minimum(text_length.astype(np.int64), min(T - 1, U)) + 1).astype(
        np.float32
    )
    in_maps = []
    for i in range(N_CORES):
        sl = slice(i * B_LOC, (i + 1) * B_LOC)
        in_maps.append(
            {
                "alpha": np.ascontiguousarray(alpha[sl]),
                "ctc": np.ascontiguousarray(ctc_log_probs[sl]),
                "mask": np.ascontiguousarray(mask[sl]).view(np.uint8),
                "consts": _make_consts(lim_full[sl], np.asarray(alpha[sl], np.float32)),
            }
        )
    return in_maps


def kernel(alpha, ctc_log_probs, mask, text_length):
    nc = _get_nc()
    in_maps = make_in_maps(alpha, ctc_log_probs, mask, text_length)
    res = run_bass_kernel_spmd(nc, in_maps, list(range(N_CORES)))
    total = np.float32(0.0)
    for r in res.results:
        total += r["out"].astype(np.float32).sum(dtype=np.float32)
    out = np.asarray(total / np.float32(B_FULL), dtype=np.float32)
    return out
